# revision 33
# baseline (speedup 1.0000x reference)
"""Channel-attention kernel for Trainium2, data-parallel over batch on 8 NeuronCores.

Reference computation (per batch b):
    xr   = x[b].reshape(HW, C)                  # [4096, 512]
    s    = xr^T @ xr                            # [C, C] gram matrix
    attn = softmax(s, axis=-1)
    v    = xr @ attn                            # [4096, 512]
    out  = beta * v + x[b]

Device strategy (per core: 2 batches, software-pipelined):
  - GEMMs in fp8e4m3 MatmulPerfMode.DoubleRow ((6+512)/2.4GHz = 216ns per
    512-col matmul): 256 matmuls -> 55.3us PE floor per core.
  - 24MB HBM traffic/core (xq fp8 natural for GEMM1, xt fp8 transposed
    k-pair for GEMM2, xn fp16 for the epilogue, fp16 out), moved in
    512KB-1MB transfers for DMA efficiency (>=1MiB reaches ~75% of
    peak; the old 256KB tiling measured 295GB/s).  HW-measured engine
    costs rule out every smaller-traffic epilogue: int8/fp8 operands or
    dst drop DVE tensor ops to 1x (2.2-5us per chunk) and Pool runs f16
    adds at 4.6us, so a quantized x or out costs more engine-seconds
    than the DMA bytes it saves.
  - Epilogue per chunk: PSUM pair tiles [128,1024] so ScalarE drains 3
    banks in 2 ops (1113+679ns), DVE drains the 4th (690ns) and does
    the only fast combine on this silicon: fp16+fp16 tensor_add at 2x
    (1217ns/[2048]).  Stores coalesce 2 chunks (1MB).
  - Load order on the sync queue is consumption priority: xq(b0) (first
    group quartered for an early GEMM1 start), xt(b0) in n-order
    halves, beta, xn(b0)/xq(b1) interleaved, xt(b1), xn(b1).
  - PE order: G1(b0) kt-major head + cb-major tail w/ early softmax,
    G2(b0) chunks with G1(b1) k-steps interleaved (3 kt per chunk) and
    softmax(b1) emitted mid-stream, then G2(b1) rotating over all 8
    PSUM banks (the gram pairs are free by then).
"""

import ml_dtypes
import numpy as np

import concourse.bass as bass
import concourse.tile as tile
from concourse import bacc, mybir
from concourse.bass_utils import run_bass_kernel_spmd

N_CORES = 8
B_FULL = 16
B_PER_CORE = B_FULL // N_CORES  # 2
H = 64
W = 64
HW = H * W  # 4096
C = 512
NT = HW // 128  # 32 row tiles
KT = NT // 2  # 16 DoubleRow k-steps (256 rows each)
CB = C // 128  # 4 channel blocks
CJ = CB // 2  # 2 channel k-pair blocks (256 channels each)
NS = NT // 4  # 8 chunks (4 row tiles each)
GSZ = 4  # row tiles per chunk
QG = 8  # row tiles per xq group (1MB fp8 tiles)
NQ = NT // QG  # 4 xq groups per batch
XNH = NS // 2  # xn tiles per batch (each covers 2 chunks, 1MB)

F32 = mybir.dt.float32
F16 = mybir.dt.float16
FP8 = mybir.dt.float8e4
AXL = mybir.AxisListType
ALU = mybir.AluOpType
ACTFN = mybir.ActivationFunctionType
DROW = mybir.MatmulPerfMode.DoubleRow

G1_TAIL = 4  # trailing k-steps emitted cb-major so softmax starts early


class BatchState:
    def __init__(self):
        self.xq = []  # fp8 [128, QG, C] group tiles (natural layout)
        self.xn = []  # f16 [128, 8, C] tiles (natural layout, 2 chunks each)
        self.xt = []  # fp8 [128, 2, HW] transposed k-pair tiles
        self.s_pair = []  # gram PSUM pair tiles [128, 2*C] (cb 2k, 2k+1)
        self.at = []  # fp8 [128, 2, C] attn k-pair tiles
        self.rsc = {}  # cb -> (exps, scale)
        self.o16 = []  # f16 [128, 8, C] output tiles (2 chunks each)
        self.vpairs = None  # current GEMM2 chunk PSUM pair tiles


def emit_loads(nc, pools, xq_ap, xn_ap, xt_ap, beta_ap, beta_sb, states):
    """All input DMAs on the sync queue in consumption-priority order."""

    qrot = [nc.sync, nc.gpsimd, nc.scalar, nc.gpsimd]

    def load_xq(b, g, pieces, dual=False):
        st = states[b]
        t = pools["xq"].tile([128, QG, C], FP8, tag="xq", name=f"xq_b{b}_g{g}")
        src = xq_ap[b, g * QG * 128 : (g + 1) * QG * 128, :].rearrange(
            "(p f) c -> p f c", p=128
        )
        step = QG // pieces
        for q in range(pieces):
            # dual: rotate issue queues (all idle this early) so the head
            # loads pipeline deeper in the DMA engines
            eng = qrot[q % 4] if dual else nc.sync
            eng.dma_start(
                t[:, q * step : (q + 1) * step, :], src[:, q * step : (q + 1) * step, :]
            )
        st.xq.append(t)

    def load_xt_piece(b, j, p):
        st = states[b]
        if len(st.xt) <= j:
            st.xt.append(
                pools["xt"].tile([128, 2, HW], FP8, tag="xt", name=f"xt_b{b}_j{j}")
            )
        n0 = p * (HW // 2)
        n1 = (p + 1) * (HW // 2)
        nc.sync.dma_start(
            st.xt[j][:, :, n0:n1],
            xt_ap[b, j, :, :].rearrange("p (i n) -> p i n", n=HW)[:, :, n0:n1],
        )

    def load_xn(b, h):
        st = states[b]
        t = pools["xn"].tile([128, 2 * GSZ, C], F16, tag="xn", name=f"xn_b{b}_h{h}")
        nc.sync.dma_start(
            t[:, :, :],
            xn_ap[b, h * 2 * GSZ * 128 : (h + 1) * 2 * GSZ * 128, :].rearrange(
                "(p f) c -> p f c", p=128
            ),
        )
        st.xn.append(t)

    load_xq(0, 0, pieces=4, dual=True)  # GEMM1 starts after 256KB lands
    load_xq(0, 1, pieces=2, dual=True)
    load_xq(0, 2, pieces=2, dual=True)
    load_xq(0, 3, pieces=2, dual=True)
    nc.sync.dma_start(beta_sb[0:1, 0:1], beta_ap[None, :])
    # GEMM1(b1) starts right after GEMM1(b0): its groups must land first;
    # then everything else strictly by first-consumption time
    load_xq(1, 0, pieces=2)
    for j in range(CJ):
        load_xt_piece(0, j, 0)
    load_xq(1, 1, pieces=2)
    load_xn(0, 0)
    load_xn(0, 1)
    for j in range(CJ):
        load_xt_piece(0, j, 1)
    load_xq(1, 2, pieces=1)
    load_xn(0, 2)
    load_xq(1, 3, pieces=1)
    load_xn(0, 3)
    for p in range(2):
        for j in range(CJ):
            load_xt_piece(1, j, p)
    for h in range(XNH):
        load_xn(1, h)


def _sps(st, cb):
    return st.s_pair[cb // 2][:, (cb % 2) * C : (cb % 2 + 1) * C]


def _g1mm(nc, st, kt, cb):
    gi, k = divmod(2 * kt, QG)
    nc.tensor.matmul(
        _sps(st, cb),
        st.xq[gi][:, k : k + 2, cb * 128 : (cb + 1) * 128],
        st.xq[gi][:, k : k + 2, :],
        start=(kt == 0),
        stop=(kt == KT - 1),
        perf_mode=DROW,
    )


def emit_g1_head(nc, pools, b, st, kts, ps):
    """kt-major GEMM1 k-steps (head)."""
    if not st.s_pair:
        st.s_pair = [
            pools[ps].tile([128, 2 * C], F32, tag=ps[-2:], name=f"s_b{b}_{i}")
            for i in range(CB // 2)
        ]
    for kt in kts:
        for cb in range(CB):
            _g1mm(nc, st, kt, cb)


def emit_g1_tail(nc, pools, beta_bc, b, st, cbs, muls=(), mul_eng="scalar"):
    """cb-major tail: each cb's softmax exp right after its last matmul."""
    for cb in cbs:
        for kt in range(KT - G1_TAIL, KT):
            _g1mm(nc, st, kt, cb)
        emit_softmax_exp(nc, pools, beta_bc, b, st, cb)
    for cb in muls:
        emit_softmax_mul(nc, b, st, cb, mul_eng)


def emit_softmax_exp(nc, pools, beta_bc, b, st, cb):
    if not st.at:
        st.at = [
            pools["at"].tile([128, 2, C], FP8, tag="at", name=f"at_b{b}_j{j}")
            for j in range(CJ)
        ]
    nmax = pools["st"].tile([128, 1], F32, tag="nmax")
    nc.vector.tensor_reduce(
        nmax[:, :], _sps(st, cb), axis=AXL.X, op=ALU.max, negate=True
    )
    exps = pools["sm"].tile([128, C], F16, tag="exps", name=f"exps_b{b}_{cb}")
    ssum = pools["st"].tile([128, 1], F32, tag="ssum")
    nc.scalar.activation(
        exps[:, :],
        _sps(st, cb),
        ACTFN.Exp,
        bias=nmax[:, :],
        scale=1.0,
        accum_out=ssum[:, :],
    )
    rinv = pools["st"].tile([128, 1], F32, tag="rinv")
    nc.vector.reciprocal(rinv[:, :], ssum[:, :])
    rsc = pools["st"].tile([128, 1], F32, tag="rsc", name=f"rsc_b{b}_{cb}")
    nc.vector.tensor_mul(rsc[:, :], rinv[:, :], beta_bc[:, :])
    st.rsc[cb] = (exps, rsc)


def emit_softmax_mul(nc, b, st, cb, eng="scalar"):
    # eng picks the queue: ScalarE when it has slack (the b0 junction),
    # DVE tensor_scalar mid-stream where ScalarE is drain-saturated
    exps, rsc = st.rsc[cb]
    if eng == "scalar":
        nc.scalar.activation(
            st.at[cb // 2][:, cb % 2, :], exps[:, :], ACTFN.Copy, scale=rsc[:, :]
        )
    else:
        nc.vector.tensor_scalar_mul(
            st.at[cb // 2][:, cb % 2, :], exps[:, :], rsc[:, :]
        )


def emit_g2_mms(nc, pools, b, s, st, ps, phase):
    """GEMM2 chunk s matmuls.  phase: 'j0' seeds all 4 banks with the j0
    accumulation (only needs the early attn pair), 'j1' finishes them,
    'full' does both.  Returns after stashing the pair tiles on st."""
    if phase in ("j0", "full"):
        st.vpairs = [
            pools[ps].tile([128, 2 * C], F32, tag=ps[-2:], name=f"v_b{b}_s{s}_{i}")
            for i in range(2)
        ]
    pairs = st.vpairs

    def vp(f):
        return pairs[f // 2][:, (f % 2) * C : (f % 2 + 1) * C]

    def mm(f, j):
        nt = GSZ * s + f
        nc.tensor.matmul(
            vp(f),
            st.xt[j][:, :, nt * 128 : (nt + 1) * 128],
            st.at[j][:, :, :],
            start=(j == 0),
            stop=(j == CJ - 1),
            perf_mode=DROW,
        )

    if phase == "j0":
        for f in range(GSZ):
            mm(f, 0)
    elif phase == "j1":
        for f in range(GSZ):
            mm(f, 1)
    else:
        for f in range(GSZ):
            for j in range(CJ):
                mm(f, j)


def emit_g2_drain(nc, pools, oh_ap, b, s, st):
    """Drains, DVE f16 add, stores on sync."""
    vc = pools["vc"].tile([128, GSZ, C], F16, tag="vc", name=f"vc_b{b}_s{s}")
    pairs = st.vpairs
    last = s == NS - 1
    if b == 1 and s == NS - 2:
        # penultimate chunk: all drains on ScalarE so DVE's queue holds
        # only adds and the epilogue backlog flushes before the PE ends
        nc.scalar.copy(
            vc[:, 0:2, :].rearrange("p f c -> p (f c)"), pairs[0][:, :]
        )
        nc.scalar.copy(
            vc[:, 2:4, :].rearrange("p f c -> p (f c)"), pairs[1][:, :]
        )
    elif b == 1 and last:
        # final chunk: 4 parallel single drains, minimal latency
        nc.scalar.copy(vc[:, 0, :], pairs[0][:, 0:C])
        nc.vector.tensor_copy(vc[:, 1, :], pairs[0][:, C : 2 * C])
        nc.scalar.copy(vc[:, 2, :], pairs[1][:, 0:C])
        nc.vector.tensor_copy(vc[:, 3, :], pairs[1][:, C : 2 * C])
    elif last:
        # b0 final chunk: 4 single drains on S/D, half adds
        nc.scalar.copy(vc[:, 0, :], pairs[0][:, 0:C])
        nc.vector.tensor_copy(vc[:, 1, :], pairs[0][:, C : 2 * C])
        nc.scalar.copy(vc[:, 2, :], pairs[1][:, 0:C])
        nc.vector.tensor_copy(vc[:, 3, :], pairs[1][:, C : 2 * C])
    else:
        # ScalarE: pairA + half of pairB; DVE: the last bank
        nc.scalar.copy(
            vc[:, 0:2, :].rearrange("p f c -> p (f c)"), pairs[0][:, :]
        )
        nc.scalar.copy(vc[:, 2, :], pairs[1][:, 0:C])
        nc.vector.tensor_copy(vc[:, 3, :], pairs[1][:, C : 2 * C])

    # epilogue: DVE f16 add into the 2-chunk output tile, store per pair
    if s % 2 == 0:
        st.o16.append(
            pools["o16"].tile([128, 2 * GSZ, C], F16, tag="o16", name=f"o16_b{b}_h{s//2}")
        )
    o16 = st.o16[s // 2]
    xn_t = st.xn[s // 2]
    lo = (s % 2) * GSZ
    spans = ((0, 2), (2, 4)) if last else ((0, 4),)
    for l, h in spans:
        nc.vector.tensor_add(
            o16[:, lo + l : lo + h, :].rearrange("p f c -> p (f c)"),
            vc[:, l:h, :].rearrange("p f c -> p (f c)"),
            xn_t[:, lo + l : lo + h, :].rearrange("p f c -> p (f c)"),
        )
    oh_rows = oh_ap[b, s // 2, :, :].rearrange("p (f c) -> p f c", c=C)
    if last:
        # store in pieces so the first bytes leave while the final
        # half-add still runs
        if b == 0:
            # b0 stores by pair: its s6 half goes out with this tile
            nc.sync.dma_start(oh_rows[:, 0:4, :], o16[:, 0:4, :])
        nc.sync.dma_start(oh_rows[:, lo : lo + 2, :], o16[:, lo : lo + 2, :])
        nc.sync.dma_start(oh_rows[:, lo + 2 : lo + 4, :], o16[:, lo + 2 : lo + 4, :])
    elif b == 1:
        # the tail batch stores per chunk to shorten the drain->store chain
        nc.sync.dma_start(oh_rows[:, lo : lo + GSZ, :], o16[:, lo : lo + GSZ, :])
    elif s % 2 == 1:
        nc.sync.dma_start(oh_rows[:, :, :], o16[:, :, :])


def channel_attention_body(tc, oh_ap, xq_ap, xn_ap, xt_ap, beta_ap):
    nc = tc.nc
    from contextlib import ExitStack

    with ExitStack() as ctx:
        ep = ctx.enter_context
        pools = {
            "xq": ep(tc.tile_pool(name="xq", bufs=2 * NQ)),
            "xn": ep(tc.tile_pool(name="xn", bufs=2 * XNH)),
            "xt": ep(tc.tile_pool(name="xt", bufs=2 * CJ)),
            "sm": ep(tc.tile_pool(name="sm", bufs=4)),
            "at": ep(tc.tile_pool(name="at", bufs=2 * CJ)),
            "st": ep(tc.tile_pool(name="st", bufs=8)),
            "vc": ep(tc.tile_pool(name="vc", bufs=6)),
            "o16": ep(tc.tile_pool(name="o16", bufs=3)),
            "const": ep(tc.tile_pool(name="const", bufs=1)),
            "ps_a": ep(tc.tile_pool(name="ps_a", bufs=2, space="PSUM")),
            "ps_b": ep(tc.tile_pool(name="ps_b", bufs=2, space="PSUM")),
        }

        beta_sb = pools["const"].tile([1, 1], F32, tag="beta")
        beta_bc = pools["const"].tile([128, 1], F32, tag="beta_bc")

        states = [BatchState() for _ in range(B_PER_CORE)]
        emit_loads(nc, pools, xq_ap, xn_ap, xt_ap, beta_ap, beta_sb, states)
        nc.gpsimd.partition_broadcast(beta_bc[:, :], beta_sb[0:1, :])

        b0, b1 = states
        # GEMM1(b0) in ps_a: kt-major head, cb-major tail + softmax(b0)
        emit_g1_head(nc, pools, 0, b0, range(KT - G1_TAIL), "ps_a")
        emit_g1_tail(nc, pools, beta_bc, 0, b0, (0, 1), muls=(0, 1))
        emit_g1_tail(nc, pools, beta_bc, 0, b0, (2, 3), muls=(2, 3))

        # GEMM1(b1) goes to ps_b, so its k-steps start with zero stall
        # right after GEMM1(b0) and pad every latency in softmax(b0) /
        # GEMM2(b0) warmup; GEMM2(b0) reuses ps_a as exps(b0) free it.
        # softmax(b1) is spread one cb per chunk to keep ScalarE under
        # its per-chunk drain budget.
        emit_g1_head(nc, pools, 1, b1, range(0, 4), "ps_b")
        emit_g2_mms(nc, pools, 0, 0, b0, "ps_a", "j0")
        emit_g1_head(nc, pools, 1, b1, range(4, 8), "ps_b")
        emit_g2_mms(nc, pools, 0, 0, b0, "ps_a", "j1")
        emit_g2_drain(nc, pools, oh_ap, 0, 0, b0)
        fill = {
            1: lambda: emit_g1_head(nc, pools, 1, b1, range(8, 10), "ps_b"),
            2: lambda: emit_g1_head(nc, pools, 1, b1, range(10, 12), "ps_b"),
            3: lambda: emit_g1_tail(nc, pools, beta_bc, 1, b1, (0,)),
            4: lambda: emit_g1_tail(
                nc, pools, beta_bc, 1, b1, (1,), muls=(0, 1), mul_eng="vector"
            ),
            5: lambda: emit_g1_tail(
                nc, pools, beta_bc, 1, b1, (2, 3), muls=(2, 3), mul_eng="vector"
            ),
        }
        for s in range(1, NS):
            if s in fill:
                fill[s]()
            # b0 s6 starts the pool alternation: both gram(b1) pairs are
            # free once exps(b1) cb0..cb3 have run (by s5)
            ps = "ps_b" if s == 6 else "ps_a"
            emit_g2_mms(nc, pools, 0, s, b0, ps, "full")
            emit_g2_drain(nc, pools, oh_ap, 0, s, b0)

        # GEMM2(b1): rotate over both pools (gram pairs all free by now)
        for s in range(NS):
            emit_g2_mms(nc, pools, 1, s, b1, "ps_b" if s % 2 == 0 else "ps_a", "full")
            emit_g2_drain(nc, pools, oh_ap, 1, s, b1)


_NC_CACHE = None


def _build():
    global _NC_CACHE
    if _NC_CACHE is not None:
        return _NC_CACHE
    nc = bacc.Bacc(
        "TRN2",
        target_bir_lowering=False,
        debug=False,
        num_devices=N_CORES,
    )
    xq_ap = nc.dram_tensor("xq", [B_PER_CORE, HW, C], FP8, kind="ExternalInput").ap()
    xn_ap = nc.dram_tensor("xn", [B_PER_CORE, HW, C], F16, kind="ExternalInput").ap()
    xt_ap = nc.dram_tensor(
        "xt", [B_PER_CORE, CJ, 128, 2 * HW], FP8, kind="ExternalInput"
    ).ap()
    beta_ap = nc.dram_tensor("beta", [1], F32, kind="ExternalInput").ap()
    oh_ap = nc.dram_tensor(
        "out", [B_PER_CORE, XNH, 128, 8 * C], F16, kind="ExternalOutput"
    ).ap()
    with tile.TileContext(nc) as tc:
        channel_attention_body(tc, oh_ap, xq_ap, xn_ap, xt_ap, beta_ap)
    nc.compile()
    _NC_CACHE = nc
    return nc


def _pack_rows(a, gsz=GSZ):
    """[B, HW, C] -> partition-blocked rows: within each gsz-row-tile group,
    row index (p, f) so each DMA partition line is contiguous."""
    bsz = a.shape[0]
    seg = a.reshape(bsz, NT // gsz, gsz, 128, C).transpose(0, 1, 3, 2, 4)
    return np.ascontiguousarray(seg.reshape(bsz, HW, C))


def _pack_xt(xr8):
    """[B, HW, C] fp8 -> [B, CJ, 128, 2*HW] k-pair transposed layout:
    xt[b, j, p, i*HW + n] = x[b, n, j*256 + i*128 + p]."""
    bsz = xr8.shape[0]
    t = xr8.transpose(0, 2, 1)  # [B, C, HW]
    t = t.reshape(bsz, CJ, 2, 128, HW).transpose(0, 1, 3, 2, 4)
    return np.ascontiguousarray(t.reshape(bsz, CJ, 128, 2 * HW))


def _unpack_out(oh):
    """[B, XNH, 128, 8*C] f16 -> [B, HW, C] fp32."""
    bsz = oh.shape[0]
    o = oh.astype(np.float32).reshape(bsz, XNH, 128, 2 * GSZ, C)
    return o.transpose(0, 1, 3, 2, 4).reshape(bsz, HW, C)


def run(x, beta, trace=False, **trace_kwargs):
    """Shard over batch, run on 8 cores, gather. Returns (out, results)."""
    x = np.asarray(x, dtype=np.float32)
    beta = np.asarray(beta, dtype=np.float32)
    assert x.shape == (B_FULL, H, W, C), x.shape
    nc = _build()
    xr = x.reshape(B_FULL, HW, C)
    xr8 = xr.astype(ml_dtypes.float8_e4m3)
    xq = _pack_rows(xr8, QG)
    xn = _pack_rows(xr.astype(np.float16), 2 * GSZ)
    xt = _pack_xt(xr8)
    in_maps = [
        {
            "xq": xq[i * B_PER_CORE : (i + 1) * B_PER_CORE],
            "xn": xn[i * B_PER_CORE : (i + 1) * B_PER_CORE],
            "xt": xt[i * B_PER_CORE : (i + 1) * B_PER_CORE],
            "beta": beta,
        }
        for i in range(N_CORES)
    ]
    res = run_bass_kernel_spmd(
        nc, in_maps, core_ids=list(range(N_CORES)), trace=trace, **trace_kwargs
    )
    out = np.concatenate(
        [_unpack_out(np.asarray(res.results[i]["out"])) for i in range(N_CORES)],
        axis=0,
    )
    return out.reshape(B_FULL, H, W, C), res


def kernel(x, beta):
    out, _ = run(x, beta, trace=False)
    return out


# revision 34
# speedup vs baseline: 1.0092x; 1.0092x over previous
"""Channel-attention kernel for Trainium2, data-parallel over batch on 8 NeuronCores.

Reference computation (per batch b):
    xr   = x[b].reshape(HW, C)                  # [4096, 512]
    s    = xr^T @ xr                            # [C, C] gram matrix
    attn = softmax(s, axis=-1)
    v    = xr @ attn                            # [4096, 512]
    out  = beta * v + x[b]

Device strategy (per core: 2 batches, software-pipelined):
  - GEMMs in fp8e4m3 MatmulPerfMode.DoubleRow ((6+512)/2.4GHz = 216ns per
    512-col matmul): 256 matmuls -> 55.3us PE floor per core.
  - 24MB HBM traffic/core (xq fp8 natural for GEMM1, xt fp8 transposed
    k-pair for GEMM2, xn fp16 for the epilogue, fp16 out), moved in
    512KB-1MB transfers for DMA efficiency (>=1MiB reaches ~75% of
    peak; the old 256KB tiling measured 295GB/s).  HW-measured engine
    costs rule out every smaller-traffic epilogue: int8/fp8 operands or
    dst drop DVE tensor ops to 1x (2.2-5us per chunk) and Pool runs f16
    adds at 4.6us, so a quantized x or out costs more engine-seconds
    than the DMA bytes it saves.
  - Epilogue per chunk: PSUM pair tiles [128,1024] so ScalarE drains 3
    banks in 2 ops (1113+679ns), DVE drains the 4th (690ns) and does
    the only fast combine on this silicon: fp16+fp16 tensor_add at 2x
    (1217ns/[2048]).  Stores coalesce 2 chunks (1MB).
  - Load order on the sync queue is consumption priority: xq(b0) (first
    group quartered for an early GEMM1 start), xt(b0) in n-order
    halves, beta, xn(b0)/xq(b1) interleaved, xt(b1), xn(b1).
  - PE order: G1(b0) kt-major head + cb-major tail w/ early softmax,
    G2(b0) chunks with G1(b1) k-steps interleaved (3 kt per chunk) and
    softmax(b1) emitted mid-stream, then G2(b1) rotating over all 8
    PSUM banks (the gram pairs are free by then).
"""

import ml_dtypes
import numpy as np

import concourse.bass as bass
import concourse.tile as tile
from concourse import bacc, mybir
from concourse.bass_utils import run_bass_kernel_spmd

N_CORES = 8
B_FULL = 16
B_PER_CORE = B_FULL // N_CORES  # 2
H = 64
W = 64
HW = H * W  # 4096
C = 512
NT = HW // 128  # 32 row tiles
KT = NT // 2  # 16 DoubleRow k-steps (256 rows each)
CB = C // 128  # 4 channel blocks
CJ = CB // 2  # 2 channel k-pair blocks (256 channels each)
NS = NT // 4  # 8 chunks (4 row tiles each)
GSZ = 4  # row tiles per chunk
QG = 8  # row tiles per xq group (1MB fp8 tiles)
NQ = NT // QG  # 4 xq groups per batch
XNH = NS // 2  # xn tiles per batch (each covers 2 chunks, 1MB)

F32 = mybir.dt.float32
F16 = mybir.dt.float16
FP8 = mybir.dt.float8e4
AXL = mybir.AxisListType
ALU = mybir.AluOpType
ACTFN = mybir.ActivationFunctionType
DROW = mybir.MatmulPerfMode.DoubleRow

G1_TAIL = 4  # trailing k-steps emitted cb-major so softmax starts early


class BatchState:
    def __init__(self):
        self.xq = []  # fp8 [128, QG, C] group tiles (natural layout)
        self.xn = []  # f16 [128, 8, C] tiles (natural layout, 2 chunks each)
        self.xt = []  # fp8 [128, 2, HW] transposed k-pair tiles
        self.s_pair = []  # gram PSUM pair tiles [128, 2*C] (cb 2k, 2k+1)
        self.at = []  # fp8 [128, 2, C] attn k-pair tiles
        self.rsc = {}  # cb -> (exps, scale)
        self.o16 = []  # f16 [128, 8, C] output tiles (2 chunks each)
        self.vpairs = None  # current GEMM2 chunk PSUM pair tiles


def emit_loads(nc, pools, xq_ap, xn_ap, xt_ap, beta_ap, beta_sb, states):
    """All input DMAs on the sync queue in consumption-priority order."""

    qrot = [nc.sync, nc.gpsimd, nc.scalar, nc.gpsimd]

    def load_xq(b, g, pieces, dual=False):
        st = states[b]
        t = pools["xq"].tile([128, QG, C], FP8, tag="xq", name=f"xq_b{b}_g{g}")
        src = xq_ap[b, g * QG * 128 : (g + 1) * QG * 128, :].rearrange(
            "(p f) c -> p f c", p=128
        )
        step = QG // pieces
        for q in range(pieces):
            # dual: rotate issue queues (all idle this early) so the head
            # loads pipeline deeper in the DMA engines
            eng = qrot[q % 4] if dual else nc.sync
            eng.dma_start(
                t[:, q * step : (q + 1) * step, :], src[:, q * step : (q + 1) * step, :]
            )
        st.xq.append(t)

    def load_xt_piece(b, j, p):
        st = states[b]
        if len(st.xt) <= j:
            st.xt.append(
                pools["xt"].tile([128, 2, HW], FP8, tag="xt", name=f"xt_b{b}_j{j}")
            )
        n0 = p * (HW // 2)
        n1 = (p + 1) * (HW // 2)
        nc.sync.dma_start(
            st.xt[j][:, :, n0:n1],
            xt_ap[b, j, :, :].rearrange("p (i n) -> p i n", n=HW)[:, :, n0:n1],
        )

    def load_xn(b, h):
        st = states[b]
        t = pools["xn"].tile([128, 2 * GSZ, C], F16, tag="xn", name=f"xn_b{b}_h{h}")
        nc.sync.dma_start(
            t[:, :, :],
            xn_ap[b, h * 2 * GSZ * 128 : (h + 1) * 2 * GSZ * 128, :].rearrange(
                "(p f) c -> p f c", p=128
            ),
        )
        st.xn.append(t)

    load_xq(0, 0, pieces=4, dual=True)  # GEMM1 starts after 256KB lands
    load_xq(0, 1, pieces=2, dual=True)
    load_xq(0, 2, pieces=2, dual=True)
    load_xq(0, 3, pieces=2, dual=True)
    nc.sync.dma_start(beta_sb[0:1, 0:1], beta_ap[None, :])
    # GEMM1(b1) starts right after GEMM1(b0): its groups must land first;
    # then everything else strictly by first-consumption time
    load_xq(1, 0, pieces=2)
    for j in range(CJ):
        load_xt_piece(0, j, 0)
    load_xq(1, 1, pieces=2)
    load_xn(0, 0)
    load_xn(0, 1)
    for j in range(CJ):
        load_xt_piece(0, j, 1)
    load_xq(1, 2, pieces=1)
    load_xn(0, 2)
    load_xq(1, 3, pieces=1)
    load_xn(0, 3)
    for p in range(2):
        for j in range(CJ):
            load_xt_piece(1, j, p)
    for h in range(XNH):
        load_xn(1, h)


def _sps(st, cb):
    return st.s_pair[cb // 2][:, (cb % 2) * C : (cb % 2 + 1) * C]


def _g1mm(nc, st, kt, cb):
    gi, k = divmod(2 * kt, QG)
    nc.tensor.matmul(
        _sps(st, cb),
        st.xq[gi][:, k : k + 2, cb * 128 : (cb + 1) * 128],
        st.xq[gi][:, k : k + 2, :],
        start=(kt == 0),
        stop=(kt == KT - 1),
        perf_mode=DROW,
    )


def emit_g1_head(nc, pools, b, st, kts, ps):
    """kt-major GEMM1 k-steps (head)."""
    if not st.s_pair:
        st.s_pair = [
            pools[ps].tile([128, 2 * C], F32, tag=ps[-2:], name=f"s_b{b}_{i}")
            for i in range(CB // 2)
        ]
    for kt in kts:
        for cb in range(CB):
            _g1mm(nc, st, kt, cb)


def emit_g1_tail(nc, pools, beta_bc, b, st, cbs, muls=(), mul_eng="scalar"):
    """cb-major tail: each cb's softmax exp right after its last matmul."""
    for cb in cbs:
        for kt in range(KT - G1_TAIL, KT):
            _g1mm(nc, st, kt, cb)
        emit_softmax_exp(nc, pools, beta_bc, b, st, cb)
    for cb in muls:
        emit_softmax_mul(nc, b, st, cb, mul_eng)


def emit_softmax_exp(nc, pools, beta_bc, b, st, cb):
    if not st.at:
        st.at = [
            pools["at"].tile([128, 2, C], FP8, tag="at", name=f"at_b{b}_j{j}")
            for j in range(CJ)
        ]
    nmax = pools["st"].tile([128, 1], F32, tag="nmax")
    nc.vector.tensor_reduce(
        nmax[:, :], _sps(st, cb), axis=AXL.X, op=ALU.max, negate=True
    )
    exps = pools["sm"].tile([128, C], F16, tag="exps", name=f"exps_b{b}_{cb}")
    ssum = pools["st"].tile([128, 1], F32, tag="ssum")
    nc.scalar.activation(
        exps[:, :],
        _sps(st, cb),
        ACTFN.Exp,
        bias=nmax[:, :],
        scale=1.0,
        accum_out=ssum[:, :],
    )
    rinv = pools["st"].tile([128, 1], F32, tag="rinv")
    nc.vector.reciprocal(rinv[:, :], ssum[:, :])
    rsc = pools["st"].tile([128, 1], F32, tag="rsc", name=f"rsc_b{b}_{cb}")
    nc.vector.tensor_mul(rsc[:, :], rinv[:, :], beta_bc[:, :])
    st.rsc[cb] = (exps, rsc)


def emit_softmax_mul(nc, b, st, cb, eng="scalar"):
    # eng picks the queue: ScalarE when it has slack (the b0 junction),
    # DVE tensor_scalar mid-stream where ScalarE is drain-saturated
    exps, rsc = st.rsc[cb]
    if eng == "scalar":
        nc.scalar.activation(
            st.at[cb // 2][:, cb % 2, :], exps[:, :], ACTFN.Copy, scale=rsc[:, :]
        )
    else:
        nc.vector.tensor_scalar_mul(
            st.at[cb // 2][:, cb % 2, :], exps[:, :], rsc[:, :]
        )


def emit_g2_mms(nc, pools, b, s, st, ps, phase):
    """GEMM2 chunk s matmuls.  phase: 'j0' seeds all 4 banks with the j0
    accumulation (only needs the early attn pair), 'j1' finishes them,
    'full' does both.  Returns after stashing the pair tiles on st."""
    if phase in ("j0", "full"):
        st.vpairs = [
            pools[ps].tile([128, 2 * C], F32, tag=ps[-2:], name=f"v_b{b}_s{s}_{i}")
            for i in range(2)
        ]
    pairs = st.vpairs

    def vp(f):
        return pairs[f // 2][:, (f % 2) * C : (f % 2 + 1) * C]

    def mm(f, j):
        nt = GSZ * s + f
        nc.tensor.matmul(
            vp(f),
            st.xt[j][:, :, nt * 128 : (nt + 1) * 128],
            st.at[j][:, :, :],
            start=(j == 0),
            stop=(j == CJ - 1),
            perf_mode=DROW,
        )

    if phase == "j0":
        for f in range(GSZ):
            mm(f, 0)
    elif phase == "j1":
        for f in range(GSZ):
            mm(f, 1)
    else:
        for f in range(GSZ):
            for j in range(CJ):
                mm(f, j)


def emit_g2_drain(nc, pools, oh_ap, b, s, st):
    """Drains, DVE f16 add, stores on sync."""
    pairs = st.vpairs
    last = s == NS - 1
    if s % 2 == 0:
        st.o16.append(
            pools["o16"].tile([128, 2 * GSZ, C], F16, tag="o16", name=f"o16_b{b}_h{s//2}")
        )
    o16 = st.o16[s // 2]
    xn_t = st.xn[s // 2]
    lo = (s % 2) * GSZ
    oh_rows = oh_ap[b, s // 2, :, :].rearrange("p (f c) -> p f c", c=C)

    if b == 1 and s >= NS - 2:
        # tail chunks: fused DVE add straight from PSUM (drain+add+convert
        # in one 1x op per pair) -- no ScalarE, shortest mm->store chain
        for i in range(2):
            nc.vector.tensor_add(
                o16[:, lo + 2 * i : lo + 2 * i + 2, :].rearrange("p f c -> p (f c)"),
                pairs[i][:, :],
                xn_t[:, lo + 2 * i : lo + 2 * i + 2, :].rearrange("p f c -> p (f c)"),
            )
            nc.sync.dma_start(
                oh_rows[:, lo + 2 * i : lo + 2 * i + 2, :],
                o16[:, lo + 2 * i : lo + 2 * i + 2, :],
            )
        return

    vc = pools["vc"].tile([128, GSZ, C], F16, tag="vc", name=f"vc_b{b}_s{s}")
    if last:
        # b0 final chunk: 4 single drains on S/D, half adds
        nc.scalar.copy(vc[:, 0, :], pairs[0][:, 0:C])
        nc.vector.tensor_copy(vc[:, 1, :], pairs[0][:, C : 2 * C])
        nc.scalar.copy(vc[:, 2, :], pairs[1][:, 0:C])
        nc.vector.tensor_copy(vc[:, 3, :], pairs[1][:, C : 2 * C])
    else:
        # ScalarE: pairA + half of pairB; DVE: the last bank
        nc.scalar.copy(
            vc[:, 0:2, :].rearrange("p f c -> p (f c)"), pairs[0][:, :]
        )
        nc.scalar.copy(vc[:, 2, :], pairs[1][:, 0:C])
        nc.vector.tensor_copy(vc[:, 3, :], pairs[1][:, C : 2 * C])

    spans = ((0, 2), (2, 4)) if last else ((0, 4),)
    for l, h in spans:
        nc.vector.tensor_add(
            o16[:, lo + l : lo + h, :].rearrange("p f c -> p (f c)"),
            vc[:, l:h, :].rearrange("p f c -> p (f c)"),
            xn_t[:, lo + l : lo + h, :].rearrange("p f c -> p (f c)"),
        )
    if last:
        # b0 stores by pair: its s6 half goes out with this tile, the
        # final half in pieces behind the half adds
        nc.sync.dma_start(oh_rows[:, 0:4, :], o16[:, 0:4, :])
        nc.sync.dma_start(oh_rows[:, lo : lo + 2, :], o16[:, lo : lo + 2, :])
        nc.sync.dma_start(oh_rows[:, lo + 2 : lo + 4, :], o16[:, lo + 2 : lo + 4, :])
    elif b == 1:
        # the tail batch stores per chunk to shorten the drain->store chain
        nc.sync.dma_start(oh_rows[:, lo : lo + GSZ, :], o16[:, lo : lo + GSZ, :])
    elif s % 2 == 1:
        nc.sync.dma_start(oh_rows[:, :, :], o16[:, :, :])


def channel_attention_body(tc, oh_ap, xq_ap, xn_ap, xt_ap, beta_ap):
    nc = tc.nc
    from contextlib import ExitStack

    with ExitStack() as ctx:
        ep = ctx.enter_context
        pools = {
            "xq": ep(tc.tile_pool(name="xq", bufs=2 * NQ)),
            "xn": ep(tc.tile_pool(name="xn", bufs=2 * XNH)),
            "xt": ep(tc.tile_pool(name="xt", bufs=2 * CJ)),
            "sm": ep(tc.tile_pool(name="sm", bufs=4)),
            "at": ep(tc.tile_pool(name="at", bufs=2 * CJ)),
            "st": ep(tc.tile_pool(name="st", bufs=8)),
            "vc": ep(tc.tile_pool(name="vc", bufs=6)),
            "o16": ep(tc.tile_pool(name="o16", bufs=3)),
            "const": ep(tc.tile_pool(name="const", bufs=1)),
            "ps_a": ep(tc.tile_pool(name="ps_a", bufs=2, space="PSUM")),
            "ps_b": ep(tc.tile_pool(name="ps_b", bufs=2, space="PSUM")),
        }

        beta_sb = pools["const"].tile([1, 1], F32, tag="beta")
        beta_bc = pools["const"].tile([128, 1], F32, tag="beta_bc")

        states = [BatchState() for _ in range(B_PER_CORE)]
        emit_loads(nc, pools, xq_ap, xn_ap, xt_ap, beta_ap, beta_sb, states)
        nc.gpsimd.partition_broadcast(beta_bc[:, :], beta_sb[0:1, :])

        b0, b1 = states
        # GEMM1(b0) in ps_a: kt-major head, cb-major tail + softmax(b0)
        emit_g1_head(nc, pools, 0, b0, range(KT - G1_TAIL), "ps_a")
        emit_g1_tail(nc, pools, beta_bc, 0, b0, (0, 1), muls=(0, 1))
        emit_g1_tail(nc, pools, beta_bc, 0, b0, (2, 3), muls=(2, 3))

        # GEMM1(b1) goes to ps_b, so its k-steps start with zero stall
        # right after GEMM1(b0) and pad every latency in softmax(b0) /
        # GEMM2(b0) warmup; GEMM2(b0) reuses ps_a as exps(b0) free it.
        # softmax(b1) is spread one cb per chunk to keep ScalarE under
        # its per-chunk drain budget.
        emit_g1_head(nc, pools, 1, b1, range(0, 4), "ps_b")
        emit_g2_mms(nc, pools, 0, 0, b0, "ps_a", "j0")
        emit_g1_head(nc, pools, 1, b1, range(4, 8), "ps_b")
        emit_g2_mms(nc, pools, 0, 0, b0, "ps_a", "j1")
        emit_g2_drain(nc, pools, oh_ap, 0, 0, b0)
        fill = {
            1: lambda: emit_g1_head(nc, pools, 1, b1, range(8, 10), "ps_b"),
            2: lambda: emit_g1_head(nc, pools, 1, b1, range(10, 12), "ps_b"),
            3: lambda: emit_g1_tail(nc, pools, beta_bc, 1, b1, (0,)),
            4: lambda: emit_g1_tail(
                nc, pools, beta_bc, 1, b1, (1,), muls=(0, 1), mul_eng="vector"
            ),
            5: lambda: emit_g1_tail(
                nc, pools, beta_bc, 1, b1, (2, 3), muls=(2, 3), mul_eng="vector"
            ),
        }
        for s in range(1, NS):
            if s in fill:
                fill[s]()
            # b0 s6 starts the pool alternation: both gram(b1) pairs are
            # free once exps(b1) cb0..cb3 have run (by s5)
            ps = "ps_b" if s == 6 else "ps_a"
            emit_g2_mms(nc, pools, 0, s, b0, ps, "full")
            emit_g2_drain(nc, pools, oh_ap, 0, s, b0)

        # GEMM2(b1): rotate over both pools (gram pairs all free by now)
        for s in range(NS):
            emit_g2_mms(nc, pools, 1, s, b1, "ps_b" if s % 2 == 0 else "ps_a", "full")
            emit_g2_drain(nc, pools, oh_ap, 1, s, b1)


_NC_CACHE = None


def _build():
    global _NC_CACHE
    if _NC_CACHE is not None:
        return _NC_CACHE
    nc = bacc.Bacc(
        "TRN2",
        target_bir_lowering=False,
        debug=False,
        num_devices=N_CORES,
    )
    xq_ap = nc.dram_tensor("xq", [B_PER_CORE, HW, C], FP8, kind="ExternalInput").ap()
    xn_ap = nc.dram_tensor("xn", [B_PER_CORE, HW, C], F16, kind="ExternalInput").ap()
    xt_ap = nc.dram_tensor(
        "xt", [B_PER_CORE, CJ, 128, 2 * HW], FP8, kind="ExternalInput"
    ).ap()
    beta_ap = nc.dram_tensor("beta", [1], F32, kind="ExternalInput").ap()
    oh_ap = nc.dram_tensor(
        "out", [B_PER_CORE, XNH, 128, 8 * C], F16, kind="ExternalOutput"
    ).ap()
    with tile.TileContext(nc) as tc:
        channel_attention_body(tc, oh_ap, xq_ap, xn_ap, xt_ap, beta_ap)
    nc.compile()
    _NC_CACHE = nc
    return nc


def _pack_rows(a, gsz=GSZ):
    """[B, HW, C] -> partition-blocked rows: within each gsz-row-tile group,
    row index (p, f) so each DMA partition line is contiguous."""
    bsz = a.shape[0]
    seg = a.reshape(bsz, NT // gsz, gsz, 128, C).transpose(0, 1, 3, 2, 4)
    return np.ascontiguousarray(seg.reshape(bsz, HW, C))


def _pack_xt(xr8):
    """[B, HW, C] fp8 -> [B, CJ, 128, 2*HW] k-pair transposed layout:
    xt[b, j, p, i*HW + n] = x[b, n, j*256 + i*128 + p]."""
    bsz = xr8.shape[0]
    t = xr8.transpose(0, 2, 1)  # [B, C, HW]
    t = t.reshape(bsz, CJ, 2, 128, HW).transpose(0, 1, 3, 2, 4)
    return np.ascontiguousarray(t.reshape(bsz, CJ, 128, 2 * HW))


def _unpack_out(oh):
    """[B, XNH, 128, 8*C] f16 -> [B, HW, C] fp32."""
    bsz = oh.shape[0]
    o = oh.astype(np.float32).reshape(bsz, XNH, 128, 2 * GSZ, C)
    return o.transpose(0, 1, 3, 2, 4).reshape(bsz, HW, C)


def run(x, beta, trace=False, **trace_kwargs):
    """Shard over batch, run on 8 cores, gather. Returns (out, results)."""
    x = np.asarray(x, dtype=np.float32)
    beta = np.asarray(beta, dtype=np.float32)
    assert x.shape == (B_FULL, H, W, C), x.shape
    nc = _build()
    xr = x.reshape(B_FULL, HW, C)
    xr8 = xr.astype(ml_dtypes.float8_e4m3)
    xq = _pack_rows(xr8, QG)
    xn = _pack_rows(xr.astype(np.float16), 2 * GSZ)
    xt = _pack_xt(xr8)
    in_maps = [
        {
            "xq": xq[i * B_PER_CORE : (i + 1) * B_PER_CORE],
            "xn": xn[i * B_PER_CORE : (i + 1) * B_PER_CORE],
            "xt": xt[i * B_PER_CORE : (i + 1) * B_PER_CORE],
            "beta": beta,
        }
        for i in range(N_CORES)
    ]
    res = run_bass_kernel_spmd(
        nc, in_maps, core_ids=list(range(N_CORES)), trace=trace, **trace_kwargs
    )
    out = np.concatenate(
        [_unpack_out(np.asarray(res.results[i]["out"])) for i in range(N_CORES)],
        axis=0,
    )
    return out.reshape(B_FULL, H, W, C), res


def kernel(x, beta):
    out, _ = run(x, beta, trace=False)
    return out


# revision 35
# speedup vs baseline: 1.0538x; 1.0442x over previous
"""Channel-attention kernel for Trainium2, data-parallel over batch on 8 NeuronCores.

Reference computation (per batch b):
    xr   = x[b].reshape(HW, C)                  # [4096, 512]
    s    = xr^T @ xr                            # [C, C] gram matrix
    attn = softmax(s, axis=-1)
    v    = xr @ attn                            # [4096, 512]
    out  = beta * v + x[b]

Device strategy (per core: 2 batches, software-pipelined):
  - GEMMs in fp8e4m3 MatmulPerfMode.DoubleRow ((6+512)/2.4GHz = 216ns per
    512-col matmul): 256 matmuls -> 55.3us PE floor per core.
  - 24MB HBM traffic/core (xq fp8 natural for GEMM1, xt fp8 transposed
    k-pair for GEMM2, xn fp16 for the epilogue, fp16 out), moved in
    512KB-1MB transfers for DMA efficiency (>=1MiB reaches ~75% of
    peak; the old 256KB tiling measured 295GB/s).  HW-measured engine
    costs rule out every smaller-traffic epilogue: int8/fp8 operands or
    dst drop DVE tensor ops to 1x (2.2-5us per chunk) and Pool runs f16
    adds at 4.6us, so a quantized x or out costs more engine-seconds
    than the DMA bytes it saves.
  - Epilogue per chunk: PSUM pair tiles [128,1024] so ScalarE drains 3
    banks in 2 ops (1113+679ns), DVE drains the 4th (690ns) and does
    the only fast combine on this silicon: fp16+fp16 tensor_add at 2x
    (1217ns/[2048]).  Stores coalesce 2 chunks (1MB).
  - Load order on the sync queue is consumption priority: xq(b0) (first
    group quartered for an early GEMM1 start), xt(b0) in n-order
    halves, beta, xn(b0)/xq(b1) interleaved, xt(b1), xn(b1).
  - PE order: G1(b0) kt-major head + cb-major tail w/ early softmax,
    G2(b0) chunks with G1(b1) k-steps interleaved (3 kt per chunk) and
    softmax(b1) emitted mid-stream, then G2(b1) rotating over all 8
    PSUM banks (the gram pairs are free by then).
"""

import ml_dtypes
import numpy as np

import concourse.bass as bass
import concourse.tile as tile
from concourse import bacc, mybir
from concourse.bass_utils import run_bass_kernel_spmd

N_CORES = 8
B_FULL = 16
B_PER_CORE = B_FULL // N_CORES  # 2
H = 64
W = 64
HW = H * W  # 4096
C = 512
NT = HW // 128  # 32 row tiles
KT = NT // 2  # 16 DoubleRow k-steps (256 rows each)
CB = C // 128  # 4 channel blocks
CJ = CB // 2  # 2 channel k-pair blocks (256 channels each)
NS = NT // 4  # 8 chunks (4 row tiles each)
GSZ = 4  # row tiles per chunk
QG = 8  # row tiles per xq group (1MB fp8 tiles)
NQ = NT // QG  # 4 xq groups per batch
XNH = NS // 2  # xn tiles per batch (each covers 2 chunks, 1MB)

F32 = mybir.dt.float32
F16 = mybir.dt.float16
FP8 = mybir.dt.float8e4
AXL = mybir.AxisListType
ALU = mybir.AluOpType
ACTFN = mybir.ActivationFunctionType
DROW = mybir.MatmulPerfMode.DoubleRow

G1_TAIL = 4  # trailing k-steps emitted cb-major so softmax starts early


class BatchState:
    def __init__(self):
        self.xq = []  # fp8 [128, QG, C] group tiles (natural layout)
        self.xn = []  # f16 [128, 8, C] tiles (natural layout, 2 chunks each)
        self.xt = []  # fp8 [128, 2, HW] transposed k-pair tiles
        self.s_pair = []  # gram PSUM pair tiles [128, 2*C] (cb 2k, 2k+1)
        self.at = []  # fp8 [128, 2, C] attn k-pair tiles
        self.rsc = {}  # cb -> (exps, scale)
        self.o16 = []  # f16 [128, 8, C] output tiles (2 chunks each)
        self.vpairs = None  # current GEMM2 chunk PSUM pair tiles


def emit_loads(nc, pools, xq_ap, xn_ap, xt_ap, beta_ap, beta_sb, states):
    """All input DMAs on the sync queue in consumption-priority order."""

    qrot = [nc.sync, nc.gpsimd, nc.scalar, nc.gpsimd]

    def load_xq(b, g, pieces, dual=False):
        st = states[b]
        t = pools["xq"].tile([128, QG, C], FP8, tag="xq", name=f"xq_b{b}_g{g}")
        src = xq_ap[b, g * QG * 128 : (g + 1) * QG * 128, :].rearrange(
            "(p f) c -> p f c", p=128
        )
        step = QG // pieces
        for q in range(pieces):
            # dual: rotate issue queues (all idle this early) so the head
            # loads pipeline deeper in the DMA engines
            eng = qrot[q % 4] if dual else nc.sync
            eng.dma_start(
                t[:, q * step : (q + 1) * step, :], src[:, q * step : (q + 1) * step, :]
            )
        st.xq.append(t)

    def load_xt_piece(b, j, p):
        st = states[b]
        if len(st.xt) <= j:
            st.xt.append(
                pools["xt"].tile([128, 2, HW], FP8, tag="xt", name=f"xt_b{b}_j{j}")
            )
        n0 = p * (HW // 2)
        n1 = (p + 1) * (HW // 2)
        nc.sync.dma_start(
            st.xt[j][:, :, n0:n1],
            xt_ap[b, j, :, :].rearrange("p (i n) -> p i n", n=HW)[:, :, n0:n1],
        )

    def load_xn(b, h):
        st = states[b]
        t = pools["xn"].tile([128, 2 * GSZ, C], F16, tag="xn", name=f"xn_b{b}_h{h}")
        nc.sync.dma_start(
            t[:, :, :],
            xn_ap[b, h * 2 * GSZ * 128 : (h + 1) * 2 * GSZ * 128, :].rearrange(
                "(p f) c -> p f c", p=128
            ),
        )
        st.xn.append(t)

    load_xq(0, 0, pieces=4, dual=True)  # GEMM1 starts after 256KB lands
    load_xq(0, 1, pieces=2, dual=True)
    load_xq(0, 2, pieces=2, dual=True)
    load_xq(0, 3, pieces=2, dual=True)
    nc.sync.dma_start(beta_sb[0:1, 0:1], beta_ap[None, :])
    # GEMM1(b1) starts right after GEMM1(b0): its groups must land first;
    # then everything else strictly by first-consumption time
    load_xq(1, 0, pieces=2)
    for j in range(CJ):
        load_xt_piece(0, j, 0)
    load_xq(1, 1, pieces=2)
    load_xn(0, 0)
    load_xn(0, 1)
    for j in range(CJ):
        load_xt_piece(0, j, 1)
    load_xq(1, 2, pieces=1)
    load_xn(0, 2)
    load_xq(1, 3, pieces=1)
    load_xn(0, 3)
    for p in range(2):
        for j in range(CJ):
            load_xt_piece(1, j, p)
    for h in range(XNH):
        load_xn(1, h)


def _sps(st, cb):
    return st.s_pair[cb // 2][:, (cb % 2) * C : (cb % 2 + 1) * C]


def _g1mm(nc, st, kt, cb):
    gi, k = divmod(2 * kt, QG)
    nc.tensor.matmul(
        _sps(st, cb),
        st.xq[gi][:, k : k + 2, cb * 128 : (cb + 1) * 128],
        st.xq[gi][:, k : k + 2, :],
        start=(kt == 0),
        stop=(kt == KT - 1),
        perf_mode=DROW,
    )


def emit_g1_head(nc, pools, b, st, kts, ps):
    """kt-major GEMM1 k-steps (head)."""
    if not st.s_pair:
        st.s_pair = [
            pools[ps].tile([128, 2 * C], F32, tag=ps[-2:], name=f"s_b{b}_{i}")
            for i in range(CB // 2)
        ]
    for kt in kts:
        for cb in range(CB):
            _g1mm(nc, st, kt, cb)


def emit_g1_tail(nc, pools, beta_bc, b, st, cbs, muls=(), mul_eng="scalar", depth=G1_TAIL):
    """cb-major tail: each cb's softmax exp right after its last matmul."""
    for cb in cbs:
        for kt in range(KT - depth, KT):
            _g1mm(nc, st, kt, cb)
        emit_softmax_exp(nc, pools, beta_bc, b, st, cb)
    for cb in muls:
        emit_softmax_mul(nc, b, st, cb, mul_eng)


def emit_softmax_exp(nc, pools, beta_bc, b, st, cb):
    if not st.at:
        st.at = [
            pools["at"].tile([128, 2, C], FP8, tag="at", name=f"at_b{b}_j{j}")
            for j in range(CJ)
        ]
    nmax = pools["st"].tile([128, 1], F32, tag="nmax")
    nc.vector.tensor_reduce(
        nmax[:, :], _sps(st, cb), axis=AXL.X, op=ALU.max, negate=True
    )
    exps = pools["sm"].tile([128, C], F16, tag="exps", name=f"exps_b{b}_{cb}")
    ssum = pools["st"].tile([128, 1], F32, tag="ssum")
    nc.scalar.activation(
        exps[:, :],
        _sps(st, cb),
        ACTFN.Exp,
        bias=nmax[:, :],
        scale=1.0,
        accum_out=ssum[:, :],
    )
    rinv = pools["st"].tile([128, 1], F32, tag="rinv")
    nc.vector.reciprocal(rinv[:, :], ssum[:, :])
    rsc = pools["st"].tile([128, 1], F32, tag="rsc", name=f"rsc_b{b}_{cb}")
    nc.vector.tensor_mul(rsc[:, :], rinv[:, :], beta_bc[:, :])
    st.rsc[cb] = (exps, rsc)


def emit_softmax_mul(nc, b, st, cb, eng="scalar"):
    # eng picks the queue: ScalarE when it has slack (the b0 junction),
    # DVE tensor_scalar mid-stream where ScalarE is drain-saturated
    exps, rsc = st.rsc[cb]
    if eng == "scalar":
        nc.scalar.activation(
            st.at[cb // 2][:, cb % 2, :], exps[:, :], ACTFN.Copy, scale=rsc[:, :]
        )
    else:
        nc.vector.tensor_scalar_mul(
            st.at[cb // 2][:, cb % 2, :], exps[:, :], rsc[:, :]
        )


def emit_g2_mms(nc, pools, b, s, st, ps, phase):
    """GEMM2 chunk s matmuls.  phase: 'j0' seeds all 4 banks with the j0
    accumulation (only needs the early attn pair), 'j1' finishes them,
    'full' does both.  Returns after stashing the pair tiles on st."""
    if phase in ("j0", "full"):
        st.vpairs = [
            pools[ps].tile([128, 2 * C], F32, tag=ps[-2:], name=f"v_b{b}_s{s}_{i}")
            for i in range(2)
        ]
    pairs = st.vpairs

    def vp(f):
        return pairs[f // 2][:, (f % 2) * C : (f % 2 + 1) * C]

    def mm(f, j):
        nt = GSZ * s + f
        nc.tensor.matmul(
            vp(f),
            st.xt[j][:, :, nt * 128 : (nt + 1) * 128],
            st.at[j][:, :, :],
            start=(j == 0),
            stop=(j == CJ - 1),
            perf_mode=DROW,
        )

    if phase == "j0":
        for f in range(GSZ):
            mm(f, 0)
    elif phase == "j1":
        for f in range(GSZ):
            mm(f, 1)
    else:
        for f in range(GSZ):
            for j in range(CJ):
                mm(f, j)


def emit_g2_drain(nc, pools, oh_ap, b, s, st):
    """Drains, DVE f16 add, stores on sync."""
    pairs = st.vpairs
    last = s == NS - 1
    if s % 2 == 0:
        st.o16.append(
            pools["o16"].tile([128, 2 * GSZ, C], F16, tag="o16", name=f"o16_b{b}_h{s//2}")
        )
    o16 = st.o16[s // 2]
    xn_t = st.xn[s // 2]
    lo = (s % 2) * GSZ
    oh_rows = oh_ap[b, s // 2, :, :].rearrange("p (f c) -> p f c", c=C)

    if b == 1 and s >= NS - 2:
        # tail chunks: fused DVE add straight from PSUM (drain+add+convert
        # in one 1x op per pair) -- no ScalarE, shortest mm->store chain
        for i in range(2):
            nc.vector.tensor_add(
                o16[:, lo + 2 * i : lo + 2 * i + 2, :].rearrange("p f c -> p (f c)"),
                pairs[i][:, :],
                xn_t[:, lo + 2 * i : lo + 2 * i + 2, :].rearrange("p f c -> p (f c)"),
            )
            nc.sync.dma_start(
                oh_rows[:, lo + 2 * i : lo + 2 * i + 2, :],
                o16[:, lo + 2 * i : lo + 2 * i + 2, :],
            )
        return

    if b == 1:
        # fused-mix: DVE adds pairA straight from PSUM while ScalarE
        # drains pairB to vc for a cheap 2x SBUF add -- S+D per chunk
        # drops from 3.7us to 2.96us
        nc.vector.tensor_add(
            o16[:, lo : lo + 2, :].rearrange("p f c -> p (f c)"),
            pairs[0][:, :],
            xn_t[:, lo : lo + 2, :].rearrange("p f c -> p (f c)"),
        )
        vc = pools["vc"].tile([128, 2, C], F16, tag="vch", name=f"vc_b{b}_s{s}")
        nc.scalar.copy(vc[:, :, :].rearrange("p f c -> p (f c)"), pairs[1][:, :])
        nc.vector.tensor_add(
            o16[:, lo + 2 : lo + 4, :].rearrange("p f c -> p (f c)"),
            vc[:, :, :].rearrange("p f c -> p (f c)"),
            xn_t[:, lo + 2 : lo + 4, :].rearrange("p f c -> p (f c)"),
        )
        nc.sync.dma_start(oh_rows[:, lo : lo + GSZ, :], o16[:, lo : lo + GSZ, :])
        return

    vc = pools["vc"].tile([128, GSZ, C], F16, tag="vc", name=f"vc_b{b}_s{s}")
    if last:
        # b0 final chunk: 4 single drains on S/D, half adds
        nc.scalar.copy(vc[:, 0, :], pairs[0][:, 0:C])
        nc.vector.tensor_copy(vc[:, 1, :], pairs[0][:, C : 2 * C])
        nc.scalar.copy(vc[:, 2, :], pairs[1][:, 0:C])
        nc.vector.tensor_copy(vc[:, 3, :], pairs[1][:, C : 2 * C])
    else:
        # ScalarE: pairA + half of pairB; DVE: the last bank
        nc.scalar.copy(
            vc[:, 0:2, :].rearrange("p f c -> p (f c)"), pairs[0][:, :]
        )
        nc.scalar.copy(vc[:, 2, :], pairs[1][:, 0:C])
        nc.vector.tensor_copy(vc[:, 3, :], pairs[1][:, C : 2 * C])

    spans = ((0, 2), (2, 4)) if last else ((0, 4),)
    for l, h in spans:
        nc.vector.tensor_add(
            o16[:, lo + l : lo + h, :].rearrange("p f c -> p (f c)"),
            vc[:, l:h, :].rearrange("p f c -> p (f c)"),
            xn_t[:, lo + l : lo + h, :].rearrange("p f c -> p (f c)"),
        )
    if last:
        # b0 stores by pair: its s6 half goes out with this tile, the
        # final half in pieces behind the half adds
        nc.sync.dma_start(oh_rows[:, 0:4, :], o16[:, 0:4, :])
        nc.sync.dma_start(oh_rows[:, lo : lo + 2, :], o16[:, lo : lo + 2, :])
        nc.sync.dma_start(oh_rows[:, lo + 2 : lo + 4, :], o16[:, lo + 2 : lo + 4, :])
    elif b == 1:
        # the tail batch stores per chunk to shorten the drain->store chain
        nc.sync.dma_start(oh_rows[:, lo : lo + GSZ, :], o16[:, lo : lo + GSZ, :])
    elif s % 2 == 1:
        nc.sync.dma_start(oh_rows[:, :, :], o16[:, :, :])


def channel_attention_body(tc, oh_ap, xq_ap, xn_ap, xt_ap, beta_ap):
    nc = tc.nc
    from contextlib import ExitStack

    with ExitStack() as ctx:
        ep = ctx.enter_context
        pools = {
            "xq": ep(tc.tile_pool(name="xq", bufs=2 * NQ)),
            "xn": ep(tc.tile_pool(name="xn", bufs=2 * XNH)),
            "xt": ep(tc.tile_pool(name="xt", bufs=2 * CJ)),
            "sm": ep(tc.tile_pool(name="sm", bufs=4)),
            "at": ep(tc.tile_pool(name="at", bufs=2 * CJ)),
            "st": ep(tc.tile_pool(name="st", bufs=8)),
            "vc": ep(tc.tile_pool(name="vc", bufs=6)),
            "o16": ep(tc.tile_pool(name="o16", bufs=3)),
            "const": ep(tc.tile_pool(name="const", bufs=1)),
            "ps_a": ep(tc.tile_pool(name="ps_a", bufs=2, space="PSUM")),
            "ps_b": ep(tc.tile_pool(name="ps_b", bufs=2, space="PSUM")),
        }

        beta_sb = pools["const"].tile([1, 1], F32, tag="beta")
        beta_bc = pools["const"].tile([128, 1], F32, tag="beta_bc")

        states = [BatchState() for _ in range(B_PER_CORE)]
        emit_loads(nc, pools, xq_ap, xn_ap, xt_ap, beta_ap, beta_sb, states)
        nc.gpsimd.partition_broadcast(beta_bc[:, :], beta_sb[0:1, :])

        b0, b1 = states
        # GEMM1(b0) in ps_a: kt-major head, cb-major tail + softmax(b0)
        emit_g1_head(nc, pools, 0, b0, range(KT - 6), "ps_a")
        emit_g1_tail(nc, pools, beta_bc, 0, b0, (0, 1), muls=(0, 1), depth=6)
        emit_g1_tail(nc, pools, beta_bc, 0, b0, (2, 3), muls=(2, 3), depth=6)

        # GEMM1(b1) goes to ps_b, so its k-steps start with zero stall
        # right after GEMM1(b0) and pad every latency in softmax(b0) /
        # GEMM2(b0) warmup; GEMM2(b0) reuses ps_a as exps(b0) free it.
        # softmax(b1) is spread one cb per chunk to keep ScalarE under
        # its per-chunk drain budget.
        emit_g1_head(nc, pools, 1, b1, range(0, 4), "ps_b")
        emit_g2_mms(nc, pools, 0, 0, b0, "ps_a", "j0")
        emit_g1_head(nc, pools, 1, b1, range(4, 8), "ps_b")
        emit_g2_mms(nc, pools, 0, 0, b0, "ps_a", "j1")
        emit_g2_drain(nc, pools, oh_ap, 0, 0, b0)
        fill = {
            1: lambda: emit_g1_head(nc, pools, 1, b1, range(8, 10), "ps_b"),
            2: lambda: emit_g1_head(nc, pools, 1, b1, range(10, 12), "ps_b"),
            3: lambda: emit_g1_tail(nc, pools, beta_bc, 1, b1, (0,)),
            4: lambda: emit_g1_tail(
                nc, pools, beta_bc, 1, b1, (1,), muls=(0, 1), mul_eng="vector"
            ),
            5: lambda: emit_g1_tail(
                nc, pools, beta_bc, 1, b1, (2, 3), muls=(2, 3), mul_eng="vector"
            ),
        }
        for s in range(1, NS):
            if s in fill:
                fill[s]()
            # b0 s6 starts the pool alternation: both gram(b1) pairs are
            # free once exps(b1) cb0..cb3 have run (by s5)
            ps = "ps_b" if s == 6 else "ps_a"
            emit_g2_mms(nc, pools, 0, s, b0, ps, "full")
            emit_g2_drain(nc, pools, oh_ap, 0, s, b0)

        # GEMM2(b1): rotate over both pools (gram pairs all free by now)
        for s in range(NS):
            emit_g2_mms(nc, pools, 1, s, b1, "ps_b" if s % 2 == 0 else "ps_a", "full")
            emit_g2_drain(nc, pools, oh_ap, 1, s, b1)


_NC_CACHE = None


def _build():
    global _NC_CACHE
    if _NC_CACHE is not None:
        return _NC_CACHE
    nc = bacc.Bacc(
        "TRN2",
        target_bir_lowering=False,
        debug=False,
        num_devices=N_CORES,
    )
    xq_ap = nc.dram_tensor("xq", [B_PER_CORE, HW, C], FP8, kind="ExternalInput").ap()
    xn_ap = nc.dram_tensor("xn", [B_PER_CORE, HW, C], F16, kind="ExternalInput").ap()
    xt_ap = nc.dram_tensor(
        "xt", [B_PER_CORE, CJ, 128, 2 * HW], FP8, kind="ExternalInput"
    ).ap()
    beta_ap = nc.dram_tensor("beta", [1], F32, kind="ExternalInput").ap()
    oh_ap = nc.dram_tensor(
        "out", [B_PER_CORE, XNH, 128, 8 * C], F16, kind="ExternalOutput"
    ).ap()
    with tile.TileContext(nc) as tc:
        channel_attention_body(tc, oh_ap, xq_ap, xn_ap, xt_ap, beta_ap)
    nc.compile()
    _NC_CACHE = nc
    return nc


def _pack_rows(a, gsz=GSZ):
    """[B, HW, C] -> partition-blocked rows: within each gsz-row-tile group,
    row index (p, f) so each DMA partition line is contiguous."""
    bsz = a.shape[0]
    seg = a.reshape(bsz, NT // gsz, gsz, 128, C).transpose(0, 1, 3, 2, 4)
    return np.ascontiguousarray(seg.reshape(bsz, HW, C))


def _pack_xt(xr8):
    """[B, HW, C] fp8 -> [B, CJ, 128, 2*HW] k-pair transposed layout:
    xt[b, j, p, i*HW + n] = x[b, n, j*256 + i*128 + p]."""
    bsz = xr8.shape[0]
    t = xr8.transpose(0, 2, 1)  # [B, C, HW]
    t = t.reshape(bsz, CJ, 2, 128, HW).transpose(0, 1, 3, 2, 4)
    return np.ascontiguousarray(t.reshape(bsz, CJ, 128, 2 * HW))


def _unpack_out(oh):
    """[B, XNH, 128, 8*C] f16 -> [B, HW, C] fp32."""
    bsz = oh.shape[0]
    o = oh.astype(np.float32).reshape(bsz, XNH, 128, 2 * GSZ, C)
    return o.transpose(0, 1, 3, 2, 4).reshape(bsz, HW, C)


def run(x, beta, trace=False, **trace_kwargs):
    """Shard over batch, run on 8 cores, gather. Returns (out, results)."""
    x = np.asarray(x, dtype=np.float32)
    beta = np.asarray(beta, dtype=np.float32)
    assert x.shape == (B_FULL, H, W, C), x.shape
    nc = _build()
    xr = x.reshape(B_FULL, HW, C)
    xr8 = xr.astype(ml_dtypes.float8_e4m3)
    xq = _pack_rows(xr8, QG)
    xn = _pack_rows(xr.astype(np.float16), 2 * GSZ)
    xt = _pack_xt(xr8)
    in_maps = [
        {
            "xq": xq[i * B_PER_CORE : (i + 1) * B_PER_CORE],
            "xn": xn[i * B_PER_CORE : (i + 1) * B_PER_CORE],
            "xt": xt[i * B_PER_CORE : (i + 1) * B_PER_CORE],
            "beta": beta,
        }
        for i in range(N_CORES)
    ]
    res = run_bass_kernel_spmd(
        nc, in_maps, core_ids=list(range(N_CORES)), trace=trace, **trace_kwargs
    )
    out = np.concatenate(
        [_unpack_out(np.asarray(res.results[i]["out"])) for i in range(N_CORES)],
        axis=0,
    )
    return out.reshape(B_FULL, H, W, C), res


def kernel(x, beta):
    out, _ = run(x, beta, trace=False)
    return out
